# revision 4
# baseline (speedup 1.0000x reference)
"""Trainium2 Bass kernel for nn_Attention_80805514707533 — v4.

v3 + software pipelining across the step boundary. ACT runs lag-1
interleaved (tanh0, tanh1, exp0, exp1, tanh2, tanh3, exp2, exp3); group
A's softmax chain (S/recip/bc/att') fills the middle of the step; group
B's tail (bc2/bc3/att'/w3/num/out) is deferred into the NEXT step's
prologue where it overlaps tanh0/tanh1 — so neither ACT nor PE ever
waits on the full normalization.

PE: 64x64 tile mode (4 concurrent tiles/round) for wi/wa/e, 2-tile
rounds for bc, (128,32) chained passes for S/num. 36 rounds/step.
ACT: 8 instr x N=1024 — the ~8.9us/step wall.
DVE: 4 att' muls + 2 recips + 2 w3 halves + 2 out rescales.
GPSIMD: 6 w halves.

Maps per quad q (local heads 4q..4q+3):
  att tile [128,2,B]: [:,0]=[att4q;att4q+1], [:,1]=[att4q+2;att4q+3]
  pre tile: [:,0]=[pre4q;pre4q+2], [:,1]=[pre4q+1;pre4q+3] (v same)
  e tile:   [:,0]=[e4q;e4q+1],     [:,1]=[e4q+2;e4q+3]     (u same)
  S rows = local head index (A: 0-7 in rSA, B: 8-15 in rSB; B's ones
  duplicate sums into rows 0-7 so the [0:16] recip read stays finite)
"""

import numpy as np
import ml_dtypes

B, T, C, H = 512, 512, 64, 128
NCORES = 8
HL = H // NCORES          # heads per core = 16
NQ = HL // 4              # quads per core = 4
PREF = 3                  # xdup DMA prefetch distance


def _build_nc(t_steps: int):
    import concourse.bass as bass
    import concourse.bacc as bacc
    import concourse.mybir as mybir
    import concourse.tile as tile
    from contextlib import ExitStack
    from concourse.dve_ops import (RECIP_APPROX_FAST_CONSTS,
                                   RECIPROCAL_APPROX_FAST)

    fp32 = mybir.dt.float32
    bf16 = mybir.dt.bfloat16
    Tanh = mybir.ActivationFunctionType.Tanh
    Exp = mybir.ActivationFunctionType.Exp
    nc = bacc.Bacc("TRN2", target_bir_lowering=False, debug=False,
                   num_devices=NCORES)

    xT_d = nc.dram_tensor("xT", [C, t_steps, B], bf16, kind="ExternalInput")
    wi_d = nc.dram_tensor("wi", [128, NQ, 64], bf16, kind="ExternalInput")
    wi2_d = nc.dram_tensor("wi2", [128, NQ, 64], bf16, kind="ExternalInput")
    wa_d = nc.dram_tensor("wa", [128, NQ, 64], bf16, kind="ExternalInput")
    wa2_d = nc.dram_tensor("wa2", [128, NQ, 64], bf16, kind="ExternalInput")
    we_d = nc.dram_tensor("we", [128, NQ, 64], bf16, kind="ExternalInput")
    we2_d = nc.dram_tensor("we2", [128, NQ, 64], bf16, kind="ExternalInput")
    on_d = nc.dram_tensor("ones16", [128, 2 * NQ, 32], bf16, kind="ExternalInput")
    sel_d = nc.dram_tensor("sel", [64, HL, 64], bf16, kind="ExternalInput")
    out_d = nc.dram_tensor("out", [t_steps, HL, B], fp32, kind="ExternalOutput")

    _rc = RECIP_APPROX_FAST_CONSTS

    with ExitStack() as ctx:
        ctx.enter_context(nc.allow_low_precision(reason="bf16 datapath"))
        tc = ctx.enter_context(tile.TileContext(nc))
        singles = ctx.enter_context(tc.tile_pool(name="singles", bufs=1))
        attpool = ctx.enter_context(tc.tile_pool(name="attp", bufs=8))
        xpool = ctx.enter_context(tc.tile_pool(name="xpool", bufs=PREF + 2))
        vpool = ctx.enter_context(tc.tile_pool(name="vpool", bufs=2))
        upool = ctx.enter_context(tc.tile_pool(name="upool", bufs=4))
        wpool = ctx.enter_context(tc.tile_pool(name="wpool", bufs=4))
        opool = ctx.enter_context(tc.tile_pool(name="opool", bufs=3))
        ps = ctx.enter_context(tc.tile_pool(name="ps", bufs=2, space="PSUM"))
        psn = ctx.enter_context(tc.tile_pool(name="psn", bufs=2, space="PSUM"))

        wi_sb = singles.tile([128, NQ, 64], bf16)
        wi2_sb = singles.tile([128, NQ, 64], bf16)
        wa_sb = singles.tile([128, NQ, 64], bf16)
        wa2_sb = singles.tile([128, NQ, 64], bf16)
        we_sb = singles.tile([128, NQ, 64], bf16)
        we2_sb = singles.tile([128, NQ, 64], bf16)
        on_sb = singles.tile([128, 2 * NQ, 32], bf16)
        sel_sb = singles.tile([64, HL, 64], bf16)
        rSA0 = singles.tile([64, B], bf16)
        rSA1 = singles.tile([64, B], bf16)
        rSB0 = singles.tile([64, B], bf16)
        rSB1 = singles.tile([64, B], bf16)
        rSA = [rSA0, rSA1]
        rSB = [rSB0, rSB1]
        for sb, d in [(wi_sb, wi_d), (wi2_sb, wi2_d), (wa_sb, wa_d),
                      (wa2_sb, wa2_d), (we_sb, we_d), (we2_sb, we2_d),
                      (on_sb, on_d), (sel_sb, sel_d)]:
            nc.sync.dma_start(out=sb, in_=d[:])
        for r in (*rSA, *rSB):
            nc.vector.memset(r, 1.0)

        att = []
        for q in range(NQ):
            a = attpool.tile([128, 2, B], bf16, tag=f"att{q}")
            nc.vector.memset(a, 1.0 / C)
            att.append(a)

        xdups = []
        for tt in range(min(PREF, t_steps)):
            xd = xpool.tile([128, B], bf16)
            nc.sync.dma_start(out=xd[0:C, :], in_=xT_d[:, tt, :])
            nc.sync.dma_start(out=xd[C:128, :], in_=xT_d[:, tt, :])
            xdups.append(xd)

        def pre_rounds(q, xdup):
            """wi round + wa round (8 tiled matmuls) -> pre PSUM tile."""
            pre = ps.tile([128, 2, B], fp32, tag="ps")
            nc.tensor.matmul(pre[0:64, 0, :], wi_sb[0:64, q, :],
                             xdup[0:64, :], start=True, stop=False)
            nc.tensor.matmul(pre[0:64, 1, :], wi_sb[64:128, q, :],
                             xdup[64:128, :], start=True, stop=False)
            nc.tensor.matmul(pre[64:128, 0, :], wi2_sb[0:64, q, :],
                             xdup[0:64, :], start=True, stop=False)
            nc.tensor.matmul(pre[64:128, 1, :], wi2_sb[64:128, q, :],
                             xdup[64:128, :], start=True, stop=False)
            aq = att[q]
            nc.tensor.matmul(pre[0:64, 0, :], wa_sb[0:64, q, :],
                             aq[0:64, 0, :], start=False, stop=True)
            nc.tensor.matmul(pre[0:64, 1, :], wa_sb[64:128, q, :],
                             aq[64:128, 0, :], start=False, stop=True)
            nc.tensor.matmul(pre[64:128, 0, :], wa2_sb[0:64, q, :],
                             aq[0:64, 1, :], start=False, stop=True)
            nc.tensor.matmul(pre[64:128, 1, :], wa2_sb[64:128, q, :],
                             aq[64:128, 1, :], start=False, stop=True)
            return pre

        def tanh_q(pre):
            v = vpool.tile([128, 2, B], bf16)
            nc.scalar.activation(v, pre, Tanh)
            return v

        def e_exp(q, v):
            e = ps.tile([128, 2, B], fp32, tag="ps")
            nc.tensor.matmul(e[0:64, 0, :], we_sb[0:64, q, :],
                             v[0:64, 0, :], start=True, stop=True)
            nc.tensor.matmul(e[0:64, 1, :], we_sb[64:128, q, :],
                             v[64:128, 0, :], start=True, stop=True)
            nc.tensor.matmul(e[64:128, 0, :], we2_sb[0:64, q, :],
                             v[0:64, 1, :], start=True, stop=True)
            nc.tensor.matmul(e[64:128, 1, :], we2_sb[64:128, q, :],
                             v[64:128, 1, :], start=True, stop=True)
            u = upool.tile([128, 2, B], bf16)
            nc.scalar.activation(u, e, Exp)
            return u

        def s_chain(grp, u_t, par):
            # col-tiled: h=0 passes -> strip 0, h=1 -> strip 32, running
            # concurrently; 2 rounds per group. All 32 cols of each strip
            # are written (replicated sums) so the recip input is finite.
            q0 = 2 * grp
            S_t = psn.tile([128, 2, B], fp32, tag="psn")
            for q in (0, 1):
                for h in (0, 1):
                    p = 4 * grp + 2 * q + h
                    nc.tensor.matmul(S_t[32 * h:32 * h + 32, 0, :],
                                     on_sb[:, p, :], u_t[q0 + q][:, h, :],
                                     start=(q == 0), stop=(q == 1))
            rS = (rSA if grp == 0 else rSB)[par]
            nc.vector._custom_dve(RECIPROCAL_APPROX_FAST, out=rS[0:64, :],
                                  in0=S_t[0:64, 0, :], s0=_rc["s0"],
                                  s1=_rc["s1"], imm2=_rc["imm2"])

        def bc_att(q, u, rS):
            """bc broadcast rounds + att' normalize mul for quad q."""
            bc = psn.tile([128, 2, B], fp32, tag="psn")
            nc.tensor.matmul(bc[0:64, 0, :], sel_sb[:, 4 * q + 0, :],
                             rS, start=True, stop=True)
            nc.tensor.matmul(bc[64:128, 0, :], sel_sb[:, 4 * q + 1, :],
                             rS, start=True, stop=True)
            nc.tensor.matmul(bc[0:64, 1, :], sel_sb[:, 4 * q + 2, :],
                             rS, start=True, stop=True)
            nc.tensor.matmul(bc[64:128, 1, :], sel_sb[:, 4 * q + 3, :],
                             rS, start=True, stop=True)
            an = attpool.tile([128, 2, B], bf16, tag=f"att{q}")
            nc.vector.tensor_mul(an, u, bc)
            att[q] = an

        def tail_mid(tp, u_prev, w_prev, x_prev, par):
            """Rest of step tp's tail: w3 + num + out (bc2/bc3 emitted
            separately at the very front of the prologue)."""
            w3 = wpool.tile([128, 2, B], bf16)
            nc.gpsimd.tensor_mul(w3[:, 0, :], u_prev[3][:, 0, :], x_prev)
            nc.vector.tensor_mul(w3[:, 1, :], u_prev[3][:, 1, :], x_prev)
            w_prev[3] = w3
            num_t = psn.tile([128, 2, B], fp32, tag="psn")
            for grp in (0, 1):
                for q in (0, 1):
                    for h in (0, 1):
                        p = 4 * grp + 2 * q + h
                        nc.tensor.matmul(num_t[32 * h:32 * h + 32, grp, :],
                                         on_sb[:, p, :],
                                         w_prev[2 * grp + q][:, h, :],
                                         start=(q == 0), stop=(q == 1))
            outbA = opool.tile([64, B], fp32)
            outbB = opool.tile([64, B], fp32)
            nc.vector.tensor_mul(outbA, num_t[0:64, 0, :], rSA[par][0:64, :])
            nc.vector.tensor_mul(outbB, num_t[0:64, 1, :], rSB[par][0:64, :])
            # rows 0-3 = heads {0,1,4,5}, rows 32-35 = {2,3,6,7} (per
            # group); host applies the permutation when gathering
            nc.sync.dma_start(out=out_d[tp, 0:4, :], in_=outbA[0:4, :])
            nc.sync.dma_start(out=out_d[tp, 4:8, :], in_=outbA[32:36, :])
            nc.sync.dma_start(out=out_d[tp, 8:12, :], in_=outbB[0:4, :])
            nc.sync.dma_start(out=out_d[tp, 12:16, :], in_=outbB[32:36, :])

        prev = None  # (u_t, w_t, xdup) of step t-1
        # pre rounds for step 0's quads 0,1 (initial att state)
        pre0 = pre_rounds(0, xdups[0])
        pre1 = pre_rounds(1, xdups[0])
        for t in range(t_steps):
            if t + PREF < t_steps:
                xd = xpool.tile([128, B], bf16)
                nc.sync.dma_start(out=xd[0:C, :], in_=xT_d[:, t + PREF, :])
                nc.sync.dma_start(out=xd[C:128, :], in_=xT_d[:, t + PREF, :])
                xdups.append(xd)
            xdup = xdups[t]

            # prologue: group B broadcast/normalize of t-1 first (gets
            # att'2/att'3 onto the DVE early for this step's pre2/pre3)
            if prev is not None:
                bc_att(2, prev[0][2], rSB[(t - 1) % 2])
                bc_att(3, prev[0][3], rSB[(t - 1) % 2])

            v0 = tanh_q(pre0)
            v1 = tanh_q(pre1)

            u_t = [None] * NQ
            w_t = [None] * NQ
            u_t[0] = e_exp(0, v0)
            u_t[1] = e_exp(1, v1)
            for q in (0, 1):
                w = wpool.tile([128, 2, B], bf16)
                nc.gpsimd.tensor_mul(w[:, 0, :], u_t[q][:, 0, :], xdup)
                nc.gpsimd.tensor_mul(w[:, 1, :], u_t[q][:, 1, :], xdup)
                w_t[q] = w

            pre2 = pre_rounds(2, xdup)
            pre3 = pre_rounds(3, xdup)

            s_chain(0, u_t, t % 2)
            if prev is not None:
                tail_mid(t - 1, *prev, (t - 1) % 2)
            bc_att(0, u_t[0], rSA[t % 2])
            bc_att(1, u_t[1], rSA[t % 2])

            v2 = tanh_q(pre2)
            v3 = tanh_q(pre3)
            u_t[2] = e_exp(2, v2)
            u_t[3] = e_exp(3, v3)
            w2 = wpool.tile([128, 2, B], bf16)
            nc.gpsimd.tensor_mul(w2[:, 0, :], u_t[2][:, 0, :], xdup)
            nc.gpsimd.tensor_mul(w2[:, 1, :], u_t[2][:, 1, :], xdup)
            w_t[2] = w2

            # pre rounds for step t+1 quads 0,1: att'0/att'1 of step t are
            # already computed; runs on PE during exp3 so tanh0(t+1) can
            # start immediately after exp3(t)
            if t + 1 < t_steps:
                pre0 = pre_rounds(0, xdups[t + 1])
                pre1 = pre_rounds(1, xdups[t + 1])

            s_chain(1, u_t, t % 2)
            prev = (u_t, w_t, xdup)

        bc_att(2, prev[0][2], rSB[(t_steps - 1) % 2])
        bc_att(3, prev[0][3], rSB[(t_steps - 1) % 2])
        tail_mid(t_steps - 1, *prev, (t_steps - 1) % 2)

    nc.compile()
    return nc


def _host_prep(x, weight_att, weight_input, weight_e):
    bf = ml_dtypes.bfloat16
    xT = np.ascontiguousarray(x.transpose(2, 1, 0)).astype(bf)  # [C, T, B]

    # col-tiled S/num: pass p=(grp,q,h) writes strip 32h, cols c with
    # c%4 == 2q+m for head 4q+2h+m (m=0 top rows / m=1 bottom); every
    # col of the strip is written so the recip input is finite
    on = np.zeros((128, 2 * NQ, 32), np.float32)
    for p in range(2 * NQ):
        grp, q, h = p // 4, (p // 2) % 2, p % 2
        for c in range(32):
            if c % 4 == 2 * q:
                on[0:64, p, c] = 1.0
            elif c % 4 == 2 * q + 1:
                on[64:128, p, c] = 1.0
    # S-bank row of local head j (within its group's bank): 32h + 2q + m
    sel = np.zeros((64, HL, 64), np.float32)
    for j in range(HL):
        jj = j % 8
        q, h, m = jj // 4, (jj % 4) // 2, jj % 2
        sel[32 * h + 2 * q + m, j, :] = 1.0

    in_maps = []
    for g in range(NCORES):
        h0 = g * HL
        wi = np.zeros((128, NQ, 64), np.float32)
        wi2 = np.zeros((128, NQ, 64), np.float32)
        wa = np.zeros((128, NQ, 64), np.float32)
        wa2 = np.zeros((128, NQ, 64), np.float32)
        we = np.zeros((128, NQ, 64), np.float32)
        we2 = np.zeros((128, NQ, 64), np.float32)
        for q in range(NQ):
            h4 = h0 + 4 * q
            wi[0:64, q] = weight_input[h4 + 0].T
            wi[64:128, q] = weight_input[h4 + 1].T
            wi2[0:64, q] = weight_input[h4 + 2].T
            wi2[64:128, q] = weight_input[h4 + 3].T
            wa[0:64, q] = weight_att[h4 + 0].T
            wa[64:128, q] = weight_att[h4 + 1].T
            wa2[0:64, q] = weight_att[h4 + 2].T
            wa2[64:128, q] = weight_att[h4 + 3].T
            we[0:64, q] = weight_e[h4 + 0].T
            we[64:128, q] = weight_e[h4 + 2].T
            we2[0:64, q] = weight_e[h4 + 1].T
            we2[64:128, q] = weight_e[h4 + 3].T
        in_maps.append({
            "xT": xT,
            "wi": wi.astype(bf), "wi2": wi2.astype(bf),
            "wa": wa.astype(bf), "wa2": wa2.astype(bf),
            "we": we.astype(bf), "we2": we2.astype(bf),
            "ones16": on.astype(bf), "sel": sel.astype(bf),
        })
    return in_maps


def run(x, weight_att, weight_input, weight_e, t_steps=T, trace=False):
    from concourse.bass_utils import run_bass_kernel_spmd

    nc = _build_nc(t_steps)
    in_maps = _host_prep(x, weight_att, weight_input, weight_e)
    if t_steps != T:
        for m in in_maps:
            m["xT"] = np.ascontiguousarray(m["xT"][:, :t_steps, :])
    res = run_bass_kernel_spmd(nc, in_maps, list(range(NCORES)), trace=trace)

    # out_d row order per group: heads [0,1,4,5,2,3,6,7]
    perm = np.array([0, 1, 4, 5, 2, 3, 6, 7] + [8 + k for k in (0, 1, 4, 5, 2, 3, 6, 7)])
    out = np.empty((B, t_steps, H), np.float32)
    for g in range(NCORES):
        og = res.results[g]["out"]
        out[:, :, g * HL + perm] = og.transpose(2, 0, 1)
    return out, res


def kernel(x, weight_att, weight_input, weight_e):
    out, _ = run(x, weight_att, weight_input, weight_e)
    return out
